# revision 49
# baseline (speedup 1.0000x reference)
"""Chamfer loss on 8 trn2 NeuronCores (Bass/Tile).

Reference computation (per batch b):
    d2[n, m] = ||pred[b,n] - target[b,m]||^2   (floored at 0)
    loss = mean_n min_m d2 + mean_m min_n d2,  averaged over batches.

Strategy (matches the data-parallel + N-tiling sharding hint):
  - 8 cores = 4 batches x 2 halves of N (rows of pred).
  - Core (b, h) computes nd2 = -d2 for its [4096 x 8192] block on the PE via
    a K=16 augmented matmul:
        nd2[n, m] = 2 p.t - |p|^2 - |t|^2
    where each fp32 coordinate/norm is hi/lo-split into a bf16 pair, so the
    bf16 matmul (full PE rate) reproduces fp32-level accuracy (~1e-5 abs).
  - Per [128, 8192] row tile: ACT casts the fp32 PSUM tiles to one bf16 SBUF
    image; DVE then does (a) row path: pairwise max fold tree 8192 -> 2048,
    streamed out per tile, and (b) col path: one elementwise max accumulate
    into colacc [128, 8192] (shipped whole at the end).
  - Host finishes the reductions (last row-fold levels, colacc partition
    fold, min across the two N-halves per batch, means) in numpy — the same
    role the sharding hint assigns to the cross-device min-reduce.
  - DVE (0.96 GHz, 2x bf16 mode, port-bound) and the ACT PSUM-evacuation
    cast are co-bottlenecks at ~250 us each; ~281 us/core on HW.
"""

import numpy as np
import ml_dtypes

B = 4
NPTS = 8192          # pred points per batch
MPTS = 8192          # target points per batch
NH = NPTS // 2       # rows per core
P = 128              # partitions
FD = 512             # matmul free dim (one PSUM bank)
CFD = 2048           # cast-group free dim (4 banks)
R_TILES = NH // P    # 32 row tiles per core
N_G = MPTS // CFD    # 4 cast groups per row tile
N_C = MPTS // FD     # 16 col chunks
K_AUG = 16           # augmented contraction dim (hi/lo compensated bf16)
GPSIMD_COL_GS = ()   # cast groups whose col-path max runs on GPSIMD
                     # (this walrus rejects TensorTensor on Pool: NCC_IXCG966)

_CACHE = {}


def _split_multi_waits(bir_json):
    """This container's walrus caps sync waits at 1 per instruction. Split any
    instruction carrying N>1 waits into N-1 single-wait NoOps (same engine,
    inserted just before it) plus the original with one wait."""
    import json

    d = json.loads(bir_json)
    count = 0
    for fn in d["functions"]:
        for blk in fn["blocks"]:
            out = []
            for ins in blk["instructions"]:
                si = ins.get("sync_info")
                waits = (si or {}).get("on_wait") or []
                if len(waits) > 1:
                    for w in waits[:-1]:
                        count += 1
                        out.append({
                            "debug": ins.get("debug", 0),
                            "engine": ins["engine"],
                            "ins": [],
                            "outs": [],
                            "name": f"waitsplit-{count}",
                            "opcode": "NoOp",
                            "sync_info": {"on_update": [], "on_wait": [w]},
                        })
                    si["on_wait"] = [waits[-1]]
                out.append(ins)
            blk["instructions"] = out
    return json.dumps(d).encode()


def _patch_compiler():
    """Route bass2jax's walrus invocation through _split_multi_waits."""
    import concourse.bass2jax as b2j

    if getattr(b2j, "_waitsplit_patched", False):
        return
    orig = b2j.compile_bir_kernel

    def patched(bir_json, *args, **kwargs):
        return orig(_split_multi_waits(bir_json), *args, **kwargs)

    b2j.compile_bir_kernel = patched
    b2j._waitsplit_patched = True


def _build_program():
    import concourse.bass as bass
    import concourse.tile as tile
    from concourse import mybir
    from contextlib import ExitStack

    _patch_compiler()

    f32 = mybir.dt.float32
    bf16 = mybir.dt.bfloat16

    nc = bass.Bass("TRN2", target_bir_lowering=False, debug=False)

    predT_d = nc.dram_tensor("predT", [K_AUG, NH], bf16, kind="ExternalInput").ap()
    targT_d = nc.dram_tensor("targT", [K_AUG, MPTS], bf16, kind="ExternalInput").ap()
    # per-row-tile row-max partials; host finishes the max. The last 8 row
    # tiles ship one fold level earlier (4096 wide) to shorten the DVE tail.
    chamx_d = nc.dram_tensor(
        "chamxw", [P, 24 * 2048], bf16, kind="ExternalOutput"
    ).ap()
    chamx2_d = nc.dram_tensor(
        "chamxw2", [P, 8 * 4096], bf16, kind="ExternalOutput"
    ).ap()
    # column accumulator dumped whole; host folds the partition axis
    chamy_d = nc.dram_tensor("colacc", [P, MPTS], bf16, kind="ExternalOutput").ap()

    with tile.TileContext(nc) as tc, ExitStack() as ctx:
        const_pool = ctx.enter_context(tc.tile_pool(name="const", bufs=1))
        acc_pool = ctx.enter_context(tc.tile_pool(name="acc", bufs=1))
        cast_pool = ctx.enter_context(tc.tile_pool(name="cast", bufs=5))
        scr_pool = ctx.enter_context(tc.tile_pool(name="scr", bufs=6))
        xout_pool = ctx.enter_context(tc.tile_pool(name="xout", bufs=4))

        predT_sb = const_pool.tile([K_AUG, NH], bf16)
        targT_sb = const_pool.tile([K_AUG, MPTS], bf16)
        # chunked loads so the first matmuls start as soon as their slice lands
        nc.sync.dma_start(predT_sb[:, :P], predT_d[:, :P])
        for g in range(N_G):
            sl = slice(g * CFD, (g + 1) * CFD)
            nc.sync.dma_start(targT_sb[:, sl], targT_d[:, sl])
        nc.sync.dma_start(predT_sb[:, P:], predT_d[:, P:])

        colacc = acc_pool.tile([P, MPTS], bf16)

        with tc.tile_pool(name="mmpsum", bufs=2, space="PSUM") as mmp:
            for r in range(R_TILES):
                lhs = predT_sb[:, r * P:(r + 1) * P]
                # full-width bf16 image of this row tile's nd2
                cast_t = colacc if r == 0 else cast_pool.tile(
                    [P, MPTS], bf16, tag="cast"
                )
                for g in range(N_G):
                    pt = mmp.tile([P, CFD], f32, tag="mm")
                    for j in range(CFD // FD):
                        off = g * CFD + j * FD
                        nc.tensor.matmul(
                            pt[:, j * FD:(j + 1) * FD],
                            lhsT=lhs,
                            rhs=targT_sb[:, off:off + FD],
                            start=True,
                            stop=True,
                        )
                    # evacuate PSUM with a dtype cast on ACT
                    nc.scalar.copy(cast_t[:, g * CFD:(g + 1) * CFD], pt[:])
                # col path: one elementwise max accumulate per row tile
                if r > 0:
                    nc.vector.tensor_tensor(
                        out=colacc[:], in0=colacc[:], in1=cast_t[:],
                        op=mybir.AluOpType.max,
                    )
                # row path: pairwise fold tree 8192 -> 2048; host finishes.
                # For the first two row tiles the first level folds per cast
                # group, so DVE work starts as soon as each ACT cast lands.
                # (Extending this to the last tile, or splitting its col
                # accumulate, regresses 4-54us — Tile scheduling is fragile.)
                if r < 2:
                    fb = scr_pool.tile([P, MPTS // 2], bf16, tag="fold4096")
                    for g in range(N_G):
                        half = CFD // 2
                        src = cast_t[:, g * CFD:(g + 1) * CFD]
                        nc.vector.tensor_tensor(
                            out=fb[:, g * half:(g + 1) * half],
                            in0=src[:, :half], in1=src[:, half:],
                            op=mybir.AluOpType.max,
                        )
                    prev = fb
                    w = MPTS // 2
                else:
                    prev = cast_t
                    w = MPTS
                while w > 4096:
                    nxt = scr_pool.tile([P, w // 2], bf16, tag=f"fold{w // 2}")
                    nc.vector.tensor_tensor(
                        out=nxt[:], in0=prev[:, :w // 2], in1=prev[:, w // 2:],
                        op=mybir.AluOpType.max,
                    )
                    prev = nxt
                    w //= 2
                if r >= 24:
                    sl = slice((r - 24) * 4096, (r - 23) * 4096)
                    nc.sync.dma_start(chamx2_d[:, sl], prev[:])
                else:
                    xout = xout_pool.tile([P, 2048], bf16, tag="xout")
                    nc.vector.tensor_tensor(
                        out=xout[:], in0=prev[:, :2048], in1=prev[:, 2048:],
                        op=mybir.AluOpType.max,
                    )
                    nc.sync.dma_start(chamx_d[:, r * 2048:(r + 1) * 2048], xout[:])

        nc.sync.dma_start(chamy_d[:], colacc[:])

    return nc


def _augment(pred_b, target_b):
    """Hi/lo-compensated bf16 augmentation so a K=16 bf16 matmul reproduces
    nd2 = 2 p.t - |p|^2 - |t|^2 to ~1e-5 absolute despite bf16 inputs.

    pred_b/target_b: [npts, 3] fp32 -> lhsT [16, n], rhs [16, m] bf16."""
    bft = ml_dtypes.bfloat16

    def hilo(x):
        h = x.astype(bft).astype(np.float32)
        l = (x - h).astype(bft).astype(np.float32)
        return h, l

    p = np.asarray(pred_b, dtype=np.float32)
    t = np.asarray(target_b, dtype=np.float32)
    ph, pl = hilo(p)
    th, tl = hilo(t)
    p2h, p2l = hilo(np.sum(p * p, axis=1))
    t2h, t2l = hilo(np.sum(t * t, axis=1))
    n, m = p.shape[0], t.shape[0]
    L = np.zeros((K_AUG, n), np.float32)
    R = np.zeros((K_AUG, m), np.float32)
    L[0:3] = 2.0 * ph.T
    R[0:3] = th.T
    L[3:6] = 2.0 * ph.T
    R[3:6] = tl.T
    L[6:9] = 2.0 * pl.T
    R[6:9] = th.T
    L[9:12] = 2.0 * pl.T
    R[9:12] = tl.T
    L[12] = p2h
    R[12] = -1.0
    L[13] = p2l
    R[13] = -1.0
    L[14] = 1.0
    R[14] = -t2h
    L[15] = 1.0
    R[15] = -t2l
    return L.astype(bft), R.astype(bft)


def kernel(pred, target):
    from concourse.bass_utils import run_bass_kernel_spmd

    pred = np.asarray(pred, dtype=np.float32)
    target = np.asarray(target, dtype=np.float32)
    assert pred.shape == (B, NPTS, 3) and target.shape == (B, MPTS, 3)

    if "nc" not in _CACHE:
        _CACHE["nc"] = _build_program()
    nc = _CACHE["nc"]

    in_maps = []
    for core in range(8):
        b, h = core // 2, core % 2
        lhs, rhs = _augment(pred[b, h * NH:(h + 1) * NH], target[b])
        in_maps.append({"predT": lhs, "targT": rhs})

    res = run_bass_kernel_spmd(nc, in_maps, list(range(8)))

    cham_x = np.empty((B, NPTS), dtype=np.float32)
    chamy_part = np.empty((B, 2, MPTS), dtype=np.float32)
    for core in range(8):
        b, h = core // 2, core % 2
        # nd2 row-max partials; n = r*128 + p
        out_x = np.asarray(res.results[core]["chamxw"], dtype=np.float32)
        out_x2 = np.asarray(res.results[core]["chamxw2"], dtype=np.float32)
        rowmax = np.concatenate([
            out_x.reshape(P, 24, 2048).max(axis=2),
            out_x2.reshape(P, 8, 4096).max(axis=2),
        ], axis=1)                                            # [p, r]
        cham_x[b, h * NH:(h + 1) * NH] = np.maximum(-rowmax.T.reshape(NH), 0.0)
        # [128, 8192] col accumulator; partition axis is the n-tile fold
        out_y = np.asarray(res.results[core]["colacc"], dtype=np.float32)
        chamy_part[b, h] = out_y.max(axis=0)
    cham_y = np.maximum(-np.max(chamy_part, axis=1), 0.0)

    loss = cham_x.mean(axis=1).mean() + cham_y.mean(axis=1).mean()
    return np.asarray(loss, dtype=np.float32)


# revision 50
# speedup vs baseline: 1.0005x; 1.0005x over previous
"""Chamfer loss on 8 trn2 NeuronCores (Bass/Tile).

Reference computation (per batch b):
    d2[n, m] = ||pred[b,n] - target[b,m]||^2   (floored at 0)
    loss = mean_n min_m d2 + mean_m min_n d2,  averaged over batches.

Strategy (matches the data-parallel + N-tiling sharding hint):
  - 8 cores = 4 batches x 2 halves of N (rows of pred).
  - Core (b, h) computes nd2 = -d2 for its [4096 x 8192] block on the PE via
    a K=16 augmented matmul:
        nd2[n, m] = 2 p.t - |p|^2 - |t|^2
    where each fp32 coordinate/norm is hi/lo-split into a bf16 pair, so the
    bf16 matmul (full PE rate) reproduces fp32-level accuracy (~1e-5 abs).
  - Per [128, 8192] row tile: ACT casts the fp32 PSUM tiles to one bf16 SBUF
    image; DVE then does (a) row path: pairwise max fold tree 8192 -> 2048,
    streamed out per tile, and (b) col path: one elementwise max accumulate
    into colacc [128, 8192] (shipped whole at the end).
  - Host finishes the reductions (last row-fold levels, colacc partition
    fold, min across the two N-halves per batch, means) in numpy — the same
    role the sharding hint assigns to the cross-device min-reduce.
  - DVE (0.96 GHz, 2x bf16 mode, port-bound) and the ACT PSUM-evacuation
    cast are co-bottlenecks at ~250 us each; ~281 us/core on HW.
"""

import numpy as np
import ml_dtypes

B = 4
NPTS = 8192          # pred points per batch
MPTS = 8192          # target points per batch
NH = NPTS // 2       # rows per core
P = 128              # partitions
FD = 512             # matmul free dim (one PSUM bank)
CFD = 2048           # cast-group free dim (4 banks)
R_TILES = NH // P    # 32 row tiles per core
N_G = MPTS // CFD    # 4 cast groups per row tile
N_C = MPTS // FD     # 16 col chunks
K_AUG = 16           # augmented contraction dim (hi/lo compensated bf16)
GPSIMD_COL_GS = ()   # cast groups whose col-path max runs on GPSIMD
                     # (this walrus rejects TensorTensor on Pool: NCC_IXCG966)

_CACHE = {}


def _split_multi_waits(bir_json):
    """This container's walrus caps sync waits at 1 per instruction. Split any
    instruction carrying N>1 waits into N-1 single-wait NoOps (same engine,
    inserted just before it) plus the original with one wait."""
    import json

    d = json.loads(bir_json)
    count = 0
    for fn in d["functions"]:
        for blk in fn["blocks"]:
            out = []
            for ins in blk["instructions"]:
                si = ins.get("sync_info")
                waits = (si or {}).get("on_wait") or []
                if len(waits) > 1:
                    for w in waits[:-1]:
                        count += 1
                        out.append({
                            "debug": ins.get("debug", 0),
                            "engine": ins["engine"],
                            "ins": [],
                            "outs": [],
                            "name": f"waitsplit-{count}",
                            "opcode": "NoOp",
                            "sync_info": {"on_update": [], "on_wait": [w]},
                        })
                    si["on_wait"] = [waits[-1]]
                out.append(ins)
            blk["instructions"] = out
    return json.dumps(d).encode()


def _patch_compiler():
    """Route bass2jax's walrus invocation through _split_multi_waits."""
    import concourse.bass2jax as b2j

    if getattr(b2j, "_waitsplit_patched", False):
        return
    orig = b2j.compile_bir_kernel

    def patched(bir_json, *args, **kwargs):
        return orig(_split_multi_waits(bir_json), *args, **kwargs)

    b2j.compile_bir_kernel = patched
    b2j._waitsplit_patched = True


def _build_program():
    import concourse.bass as bass
    import concourse.tile as tile
    from concourse import mybir
    from contextlib import ExitStack

    _patch_compiler()

    f32 = mybir.dt.float32
    bf16 = mybir.dt.bfloat16

    nc = bass.Bass("TRN2", target_bir_lowering=False, debug=False)

    predT_d = nc.dram_tensor("predT", [K_AUG, NH], bf16, kind="ExternalInput").ap()
    targT_d = nc.dram_tensor("targT", [K_AUG, MPTS], bf16, kind="ExternalInput").ap()
    # per-row-tile 1024-wide row-max partials; host finishes the max
    chamx_d = nc.dram_tensor(
        "chamxw", [P, R_TILES * 2048], bf16, kind="ExternalOutput"
    ).ap()
    # column accumulator dumped whole; host folds the partition axis
    chamy_d = nc.dram_tensor("colacc", [P, MPTS], bf16, kind="ExternalOutput").ap()

    with tile.TileContext(nc) as tc, ExitStack() as ctx:
        const_pool = ctx.enter_context(tc.tile_pool(name="const", bufs=1))
        acc_pool = ctx.enter_context(tc.tile_pool(name="acc", bufs=1))
        cast_pool = ctx.enter_context(tc.tile_pool(name="cast", bufs=5))
        scr_pool = ctx.enter_context(tc.tile_pool(name="scr", bufs=6))
        xout_pool = ctx.enter_context(tc.tile_pool(name="xout", bufs=4))

        predT_sb = const_pool.tile([K_AUG, NH], bf16)
        targT_sb = const_pool.tile([K_AUG, MPTS], bf16)
        # chunked loads so the first matmuls start as soon as their slice lands
        nc.sync.dma_start(predT_sb[:, :P], predT_d[:, :P])
        for g in range(N_G):
            sl = slice(g * CFD, (g + 1) * CFD)
            nc.sync.dma_start(targT_sb[:, sl], targT_d[:, sl])
        nc.sync.dma_start(predT_sb[:, P:], predT_d[:, P:])

        colacc = acc_pool.tile([P, MPTS], bf16)

        with tc.tile_pool(name="mmpsum", bufs=2, space="PSUM") as mmp:
            for r in range(R_TILES):
                lhs = predT_sb[:, r * P:(r + 1) * P]
                # full-width bf16 image of this row tile's nd2
                cast_t = colacc if r == 0 else cast_pool.tile(
                    [P, MPTS], bf16, tag="cast"
                )
                for g in range(N_G):
                    pt = mmp.tile([P, CFD], f32, tag="mm")
                    for j in range(CFD // FD):
                        off = g * CFD + j * FD
                        nc.tensor.matmul(
                            pt[:, j * FD:(j + 1) * FD],
                            lhsT=lhs,
                            rhs=targT_sb[:, off:off + FD],
                            start=True,
                            stop=True,
                        )
                    # evacuate PSUM with a dtype cast on ACT
                    nc.scalar.copy(cast_t[:, g * CFD:(g + 1) * CFD], pt[:])
                # col path: one elementwise max accumulate per row tile
                if r > 0:
                    nc.vector.tensor_tensor(
                        out=colacc[:], in0=colacc[:], in1=cast_t[:],
                        op=mybir.AluOpType.max,
                    )
                # row path: pairwise fold tree 8192 -> 2048; host finishes.
                # For the first two row tiles the first level folds per cast
                # group, so DVE work starts as soon as each ACT cast lands.
                # (Extending this to the last tile, or splitting its col
                # accumulate, regresses 4-54us — Tile scheduling is fragile.)
                if r < 2:
                    fb = scr_pool.tile([P, MPTS // 2], bf16, tag="fold4096")
                    for g in range(N_G):
                        half = CFD // 2
                        src = cast_t[:, g * CFD:(g + 1) * CFD]
                        nc.vector.tensor_tensor(
                            out=fb[:, g * half:(g + 1) * half],
                            in0=src[:, :half], in1=src[:, half:],
                            op=mybir.AluOpType.max,
                        )
                    prev = fb
                    w = MPTS // 2
                else:
                    prev = cast_t
                    w = MPTS
                while w > 4096:
                    nxt = scr_pool.tile([P, w // 2], bf16, tag=f"fold{w // 2}")
                    nc.vector.tensor_tensor(
                        out=nxt[:], in0=prev[:, :w // 2], in1=prev[:, w // 2:],
                        op=mybir.AluOpType.max,
                    )
                    prev = nxt
                    w //= 2
                xout = xout_pool.tile([P, 2048], bf16, tag="xout")
                nc.vector.tensor_tensor(
                    out=xout[:], in0=prev[:, :2048], in1=prev[:, 2048:],
                    op=mybir.AluOpType.max,
                )
                nc.sync.dma_start(chamx_d[:, r * 2048:(r + 1) * 2048], xout[:])

        nc.sync.dma_start(chamy_d[:], colacc[:])

    return nc


def _augment(pred_b, target_b):
    """Hi/lo-compensated bf16 augmentation so a K=16 bf16 matmul reproduces
    nd2 = 2 p.t - |p|^2 - |t|^2 to ~1e-5 absolute despite bf16 inputs.

    pred_b/target_b: [npts, 3] fp32 -> lhsT [16, n], rhs [16, m] bf16."""
    bft = ml_dtypes.bfloat16

    def hilo(x):
        h = x.astype(bft).astype(np.float32)
        l = (x - h).astype(bft).astype(np.float32)
        return h, l

    p = np.asarray(pred_b, dtype=np.float32)
    t = np.asarray(target_b, dtype=np.float32)
    ph, pl = hilo(p)
    th, tl = hilo(t)
    p2h, p2l = hilo(np.sum(p * p, axis=1))
    t2h, t2l = hilo(np.sum(t * t, axis=1))
    n, m = p.shape[0], t.shape[0]
    L = np.zeros((K_AUG, n), np.float32)
    R = np.zeros((K_AUG, m), np.float32)
    L[0:3] = 2.0 * ph.T
    R[0:3] = th.T
    L[3:6] = 2.0 * ph.T
    R[3:6] = tl.T
    L[6:9] = 2.0 * pl.T
    R[6:9] = th.T
    L[9:12] = 2.0 * pl.T
    R[9:12] = tl.T
    L[12] = p2h
    R[12] = -1.0
    L[13] = p2l
    R[13] = -1.0
    L[14] = 1.0
    R[14] = -t2h
    L[15] = 1.0
    R[15] = -t2l
    return L.astype(bft), R.astype(bft)


def kernel(pred, target):
    from concourse.bass_utils import run_bass_kernel_spmd

    pred = np.asarray(pred, dtype=np.float32)
    target = np.asarray(target, dtype=np.float32)
    assert pred.shape == (B, NPTS, 3) and target.shape == (B, MPTS, 3)

    if "nc" not in _CACHE:
        _CACHE["nc"] = _build_program()
    nc = _CACHE["nc"]

    in_maps = []
    for core in range(8):
        b, h = core // 2, core % 2
        lhs, rhs = _augment(pred[b, h * NH:(h + 1) * NH], target[b])
        in_maps.append({"predT": lhs, "targT": rhs})

    res = run_bass_kernel_spmd(nc, in_maps, list(range(8)))

    cham_x = np.empty((B, NPTS), dtype=np.float32)
    chamy_part = np.empty((B, 2, MPTS), dtype=np.float32)
    for core in range(8):
        b, h = core // 2, core % 2
        # [128, 32, 2048] nd2 row-max partials; n = r*128 + p
        out_x = np.asarray(res.results[core]["chamxw"], dtype=np.float32)
        rowmax = out_x.reshape(P, R_TILES, 2048).max(axis=2)  # [p, r]
        cham_x[b, h * NH:(h + 1) * NH] = np.maximum(-rowmax.T.reshape(NH), 0.0)
        # [128, 8192] col accumulator; partition axis is the n-tile fold
        out_y = np.asarray(res.results[core]["colacc"], dtype=np.float32)
        chamy_part[b, h] = out_y.max(axis=0)
    cham_y = np.maximum(-np.max(chamy_part, axis=1), 0.0)

    loss = cham_x.mean(axis=1).mean() + cham_y.mean(axis=1).mean()
    return np.asarray(loss, dtype=np.float32)
